# revision 10
# baseline (speedup 1.0000x reference)
"""DifferenceAwareAggregator — Bass/Tile kernel, data-parallel on 8 NeuronCores.

Per the sharding hint: batch dim (B=8192) split across 8 cores, weights
replicated. Each core runs a fused Bass kernel:

  pre  = h_n @ (W1t+W1b) - h_c @ W1b (+ b1)        (concat fold)
  hn   = gelu(LN(pre))
  G    = Wk folded into Q per head: G[b,j,h] = sum_d Wk[j,hd] Q[b,hd]
  sc   = hn . G (contract j) + additive penalty mask; softmax over n
         (denominator from the Exp activation's accumulate port)
  U    = sum_n attn*hn ;  ctxT = Wv_head^T U^T (+bv) ;  out = ctx @ Wo + bo

Row-tile = 128 rows = 4 b x 32 n; group = 16 b; quad = 64 b. Contracted dims
live on partitions; all transposes use the xbar DMA transpose (bf16).
"""

from contextlib import ExitStack

import numpy as np
import ml_dtypes

import concourse.bass as bass
import concourse.tile as tile
from concourse import bacc, mybir
from concourse._compat import with_exitstack
from concourse.bass_utils import run_bass_kernel_spmd

F32 = mybir.dt.float32
BF16 = mybir.dt.bfloat16
AF = mybir.ActivationFunctionType
OP = mybir.AluOpType
P = 128
LN_EPS = 1e-5
N_CORES = 8


def make_kernel(B_local: int, trivial_affine: bool = True,
                gelu_fused: bool = True):
    """gelu_fused=True uses the HW Gelu LUT (not simulatable in CoreSim);
    False composes x*sigmoid(1.702x) from sim-supported primitives."""
    assert B_local % 64 == 0
    NB = B_local // 128  # b-tiles
    NG = B_local // 16   # groups

    @with_exitstack
    def kernel(ctx: ExitStack, tc: tile.TileContext, outs, ins):
        nc = tc.nc
        x = ins["x"]          # [B,32,512] f32
        hc = ins["hc"]        # [B,512] f32
        pen = ins["pen"]      # [NG,128,512] f32
        w1s = ins["w1s"]      # [4,128,512] bf16
        nw1b = ins["nw1b"]    # [4,128,512] bf16
        wq = ins["wq"]        # [4,128,512] bf16
        wkt = ins["wkt"]      # [4,128,512] bf16 (Wk^T, k-tiled by (h,d))
        wv = ins["wv"]        # [4,128,512] bf16 (k-tiled by j)
        wo = ins["wo"]        # [4,128,512] bf16 (k-tiled by (h,d))
        s32 = ins["s32"]      # [32,128,128] bf16
        bvt = ins["bvt"]      # [4,128,1] f32
        bob = ins["bob"]      # [128,512] f32
        out = outs["out"]     # [B,512] f32
        if not trivial_affine:
            b1b, bqb = ins["b1b"], ins["bqb"]
            lgb, lbb = ins["lgb"], ins["lbb"]

        const = ctx.enter_context(tc.tile_pool(name="const", bufs=1))
        pa = ctx.enter_context(tc.tile_pool(name="stage_a", bufs=2))
        pg = ctx.enter_context(tc.tile_pool(name="g_sb", bufs=2))
        px = ctx.enter_context(tc.tile_pool(name="x_tiles", bufs=3))
        ph = ctx.enter_context(tc.tile_pool(name="hn_tiles", bufs=8))
        phT = ctx.enter_context(tc.tile_pool(name="hnT", bufs=8))
        pat = ctx.enter_context(tc.tile_pool(name="attn", bufs=2))
        pu = ctx.enter_context(tc.tile_pool(name="u_tiles", bufs=2))
        pq = ctx.enter_context(tc.tile_pool(name="quad", bufs=2))
        pv = ctx.enter_context(tc.tile_pool(name="smalls", bufs=8))
        psA = ctx.enter_context(tc.tile_pool(name="psA", bufs=2, space="PSUM"))
        psPre = ctx.enter_context(tc.tile_pool(name="psPre", bufs=2, space="PSUM"))
        psSc = ctx.enter_context(tc.tile_pool(name="psSc", bufs=1, space="PSUM"))
        psU = ctx.enter_context(tc.tile_pool(name="psU", bufs=1, space="PSUM"))
        psQ = ctx.enter_context(tc.tile_pool(name="psQ", bufs=1, space="PSUM"))

        # ---- persistent weights/constants ----
        def load4(ap, tag, dt=BF16, n=4, w=512):
            t = const.tile([P, n, w], dt, tag=tag)
            for k in range(n):
                nc.gpsimd.dma_start(out=t[:, k, :], in_=ap[k])
            return t

        w1s_t = load4(w1s, "w1s")
        nw1b_t = load4(nw1b, "nw1b")
        wq_t = load4(wq, "wq")
        wkt_t = load4(wkt, "wkt")
        wv_t = load4(wv, "wv")
        wo_t = load4(wo, "wo")
        bvt_t = load4(bvt, "bvt", dt=F32, w=1)
        s32_t = const.tile([P, 32, P], BF16, tag="s32")
        for t_ in range(32):
            nc.gpsimd.dma_start(out=s32_t[:, t_, :], in_=s32[t_])
        bob_t = const.tile([P, 512], F32, tag="bob")
        nc.gpsimd.dma_start(out=bob_t, in_=bob[:])
        eps_t = const.tile([P, 1], F32, tag="eps")
        nc.vector.memset(eps_t, LN_EPS)
        if not trivial_affine:
            b1b_t = const.tile([P, 512], F32, tag="b1b")
            nc.gpsimd.dma_start(out=b1b_t, in_=b1b[:])
            bqb_t = const.tile([P, 512], F32, tag="bqb")
            nc.gpsimd.dma_start(out=bqb_t, in_=bqb[:])
            lgb_t = const.tile([P, 512], F32, tag="lgb")
            nc.gpsimd.dma_start(out=lgb_t, in_=lgb[:])
            lbb_t = const.tile([P, 512], F32, tag="lbb")
            nc.gpsimd.dma_start(out=lbb_t, in_=lbb[:])

        xbar = [nc.sync, nc.sync]

        uT_quad = None
        uq_idx = 0
        for ib in range(NB):
            # ================= stage A (per 128 b's) =================
            hc_f = pa.tile([P, 512], F32, tag="hc_f")
            nc.gpsimd.dma_start(out=hc_f, in_=hc[ib * P:(ib + 1) * P])
            hc_b = pa.tile([P, 512], BF16, tag="hc_b")
            nc.gpsimd.tensor_copy(out=hc_b, in_=hc_f)
            hcT = pa.tile([P, 4, P], BF16, tag="hcT")
            nc.sync.dma_start_transpose(hcT, hc_b)

            negz_ps = psA.tile([P, 512], F32, tag="psA")
            for k in range(4):
                nc.tensor.matmul(negz_ps, lhsT=hcT[:, k, :], rhs=nw1b_t[:, k, :],
                                 start=(k == 0), stop=(k == 3))
            negz = pa.tile([P, 512], BF16, tag="negz")
            if trivial_affine:
                nc.scalar.activation(negz, negz_ps, AF.Copy)
            else:
                nc.vector.tensor_add(negz, negz_ps, b1b_t)

            q_ps = psA.tile([P, 512], F32, tag="psA")
            for k in range(4):
                nc.tensor.matmul(q_ps, lhsT=hcT[:, k, :], rhs=wq_t[:, k, :],
                                 start=(k == 0), stop=(k == 3))
            q_b = pa.tile([P, 512], BF16, tag="q_b")
            if trivial_affine:
                nc.scalar.activation(q_b, q_ps, AF.Copy)
            else:
                nc.vector.tensor_add(q_b, q_ps, bqb_t)
            qT = pa.tile([P, 4, P], BF16, tag="qT")
            nc.sync.dma_start_transpose(qT, q_b)

            # G[b,j,h]: per head, G_h = WkT_head^T @ QT_head -> [j, b]
            gb = pg.tile([P, 4, P, 8], BF16, tag="gb")  # (j-part, jt, b, h)
            for h in range(8):
                g_ps = psA.tile([P, 4, P], F32, tag="psA")
                po = 64 * (h % 2)
                kk = h // 2
                for jm in range(4):
                    nc.tensor.matmul(
                        g_ps[:, jm, :],
                        lhsT=wkt_t[po:po + 64, kk, jm * P:(jm + 1) * P],
                        rhs=qT[po:po + 64, kk, :],
                        start=True, stop=True)
                nc.vector.tensor_copy(out=gb[:, :, :, h], in_=g_ps)

            for gl in range(NG // NB):  # groups within this b-tile
                g = ib * (NG // NB) + gl
                # ============== stage B (per 16 b's) ==============
                hn_tiles = []
                hnT_tiles = []
                for t4 in range(4):
                    rt_local = gl * 4 + t4
                    b0 = ib * P + rt_local * 4
                    x_f = px.tile([P, 512], F32, tag="x_f")
                    nc.gpsimd.dma_start(
                        out=x_f, in_=x[b0:b0 + 4].flatten_outer_dims())
                    x_b = px.tile([P, 512], BF16, tag="x_b")
                    nc.gpsimd.tensor_copy(out=x_b, in_=x_f)
                    xT = px.tile([P, 4, P], BF16, tag="xT")
                    xbar[t4 % 2].dma_start_transpose(xT, x_b)

                    pre_ps = psPre.tile([P, 512], F32, tag="pre")
                    for k in range(4):
                        nc.tensor.matmul(pre_ps, lhsT=xT[:, k, :],
                                         rhs=w1s_t[:, k, :],
                                         start=(k == 0), stop=False)
                    nc.tensor.matmul(pre_ps, lhsT=s32_t[:, rt_local, :],
                                     rhs=negz, start=False, stop=True)

                    stats = pv.tile([P, 6], F32, tag="stats")
                    nc.vector.bn_stats(stats, pre_ps)
                    mv = pv.tile([P, 2], F32, tag="mv")
                    nc.vector.bn_aggr(mv, stats)
                    sd = pv.tile([P, 1], F32, tag="sd")
                    nc.scalar.activation(sd, mv[:, 1:2], AF.Sqrt, bias=eps_t)
                    rstd = pv.tile([P, 1], F32, tag="rstd")
                    nc.vector.reciprocal(rstd, sd)
                    hn = ph.tile([P, 512], BF16, tag="hn")
                    if trivial_affine and gelu_fused:
                        nmr = pv.tile([P, 1], F32, tag="nmr")
                        nc.vector.tensor_scalar(
                            out=nmr, in0=mv[:, 0:1], scalar1=rstd, scalar2=-1.0,
                            op0=OP.mult, op1=OP.mult)
                        nc.scalar.activation(hn, pre_ps, AF.Gelu,
                                             bias=nmr, scale=rstd)
                    else:
                        xh = px.tile([P, 512], F32, tag="xh")
                        nc.vector.tensor_scalar(
                            out=xh, in0=pre_ps, scalar1=mv[:, 0:1],
                            scalar2=rstd, op0=OP.subtract, op1=OP.mult)
                        if not trivial_affine:
                            nc.vector.scalar_tensor_tensor(
                                out=xh, in0=xh, scalar=1.0, in1=lgb_t,
                                op0=OP.mult, op1=OP.mult)
                            nc.vector.tensor_add(xh, xh, lbb_t)
                        if gelu_fused:
                            nc.scalar.activation(hn, xh, AF.Gelu)
                        else:
                            sg = px.tile([P, 512], F32, tag="sg")
                            nc.scalar.activation(sg, xh, AF.Sigmoid,
                                                 scale=1.702)
                            nc.vector.tensor_mul(hn, xh, sg)
                    hn_tiles.append(hn)
                    # xbar transpose needs a contiguous destination tile
                    hnT_t = phT.tile([P, 4, P], BF16, tag="hnT")
                    xbar[(t4 + 1) % 2].dma_start_transpose(hnT_t, hn)
                    hnT_tiles.append(hnT_t)

                # scores: [(16b,8h), 512 rows]. One column-block per t4,
                # accumulated start->stop BEFORE the next block starts (a
                # start clears has_written for the whole bank, but finished
                # blocks' values survive — only interleaving groups breaks).
                sc_ps = psSc.tile([P, 512], F32, tag="sc")
                glhs = gb[:, :, gl * 16:gl * 16 + 16, :]
                for t4 in range(4):
                    for k in range(4):
                        nc.tensor.matmul(
                            sc_ps[:, t4 * P:(t4 + 1) * P],
                            lhsT=glhs[:, k, :, :],
                            rhs=hnT_tiles[t4][:, k, :],
                            start=(k == 0), stop=(k == 3),
                            skip_group_check=True)
                pen_t = pat.tile([P, 512], F32, tag="pen")
                nc.gpsimd.dma_start(out=pen_t, in_=pen[g])
                nc.vector.tensor_add(sc_ps, sc_ps, pen_t)
                e_t = pat.tile([P, 512], BF16, tag="e")
                rs = pv.tile([P, 1], F32, tag="rs")
                nc.scalar.activation(e_t, sc_ps, AF.Exp, scale=0.125,
                                     accum_out=rs)
                rcp = pv.tile([P, 1], F32, tag="rcp")
                nc.vector.reciprocal(rcp, rs)
                attn = pat.tile([P, 512], BF16, tag="attn")
                nc.gpsimd.tensor_scalar_mul(out=attn, in0=e_t, scalar1=rcp)
                attnT = pat.tile([P, 4, P], BF16, tag="attnT")
                xbar[g % 2].dma_start_transpose(attnT, attn)

                u_ps = psU.tile([P, 512], F32, tag="u")
                for k in range(4):
                    nc.tensor.matmul(u_ps, lhsT=attnT[:, k, :],
                                     rhs=hn_tiles[k],
                                     start=(k == 0), stop=(k == 3))
                u_sb = pu.tile([P, 512], BF16, tag="u_sb")
                nc.scalar.activation(u_sb, u_ps, AF.Copy)
                if uT_quad is None or g % 4 == 0:
                    uT_quad = pq.tile([P, 4, 4, P], BF16, tag="uT")
                    uq_idx = g // 4
                uT_g = pu.tile([P, 4, P], BF16, tag="uT_g")
                xbar[(g + 1) % 2].dma_start_transpose(uT_g, u_sb)
                nc.vector.tensor_copy(out=uT_quad[:, :, g % 4, :], in_=uT_g)

                if g % 4 == 3:
                    # ============ quad stage (64 b's) ============
                    ctxT_ps = psQ.tile([P, 4, 64], F32, tag="ctxT")
                    uT_h = uT_quad.rearrange("p k q (b h) -> p k (q b) h", h=8)
                    for h in range(8):
                        po = 64 * (h % 2)
                        kk = h // 2
                        for k in range(4):
                            nc.tensor.matmul(
                                ctxT_ps[po:po + 64, kk, :],
                                lhsT=wv_t[:, k, h * 64:(h + 1) * 64],
                                rhs=uT_h[:, k, :, h],
                                start=(k == 0), stop=(k == 3))
                    ctxT = pq.tile([P, 4, 64], BF16, tag="ctxT_sb")
                    for kk in range(4):
                        nc.vector.tensor_scalar_add(
                            out=ctxT[:, kk, :], in0=ctxT_ps[:, kk, :],
                            scalar1=bvt_t[:, kk, :])
                    out_ps = psQ.tile([64, 512], F32, tag="outp")
                    for k in range(4):
                        nc.tensor.matmul(out_ps, lhsT=ctxT[:, k, :],
                                         rhs=wo_t[:, k, :],
                                         start=(k == 0), stop=(k == 3))
                    out_sb = pq.tile([64, 512], F32, tag="out_sb")
                    nc.vector.tensor_add(out_sb, out_ps, bob_t[0:64, :])
                    q4 = uq_idx
                    nc.sync.dma_start(out=out[q4 * 64:(q4 + 1) * 64, :],
                                      in_=out_sb)

    return kernel


def prep_inputs(h_center, h_neighbors, W1, b1, ln_g, ln_b, Wq, bq, Wk, bk,
                Wv, bv, Wo, bo, neighbor_mask, n_cores=N_CORES):
    """Host-side prep: shard data, fold weights, build constants."""
    B, N, H = h_neighbors.shape
    Bl = B // n_cores
    NGl = Bl // 16
    f32 = np.float32
    bf16 = ml_dtypes.bfloat16

    W1 = np.asarray(W1, f32)
    w1s_ = np.ascontiguousarray((W1[:H] + W1[H:]).astype(bf16).reshape(4, 128, H))
    nw1b_ = np.ascontiguousarray((-W1[H:]).astype(bf16).reshape(4, 128, H))
    wq_ = np.ascontiguousarray(np.asarray(Wq, f32).astype(bf16).reshape(4, 128, H))
    wkt_ = np.ascontiguousarray(
        np.asarray(Wk, f32).T.astype(bf16).reshape(4, 128, H))
    wv_ = np.ascontiguousarray(np.asarray(Wv, f32).astype(bf16).reshape(4, 128, H))
    wo_ = np.ascontiguousarray(np.asarray(Wo, f32).astype(bf16).reshape(4, 128, H))
    s32 = np.zeros((32, 128, 128), f32)
    s32[np.arange(32)[:, None], 4 * np.arange(32)[:, None] +
        np.arange(128)[None, :] // 32, np.arange(128)[None, :]] = 1.0
    s32 = np.ascontiguousarray(s32.astype(bf16))
    bvt = np.ascontiguousarray(np.asarray(bv, f32).reshape(4, 128, 1))
    bob = np.ascontiguousarray(np.broadcast_to(np.asarray(bo, f32), (128, H)))

    trivial = (not np.any(np.asarray(b1)) and not np.any(np.asarray(bq))
               and np.all(np.asarray(ln_g) == 1.0)
               and not np.any(np.asarray(ln_b)))

    mask = np.asarray(neighbor_mask)
    h_neighbors = np.asarray(h_neighbors, f32)
    h_center = np.asarray(h_center, f32)
    in_maps = []
    for c in range(n_cores):
        sl = slice(c * Bl, (c + 1) * Bl)
        mc = mask[sl]
        pen = np.full((NGl, 128, 512), -30000.0, f32)
        for bl in range(16):
            rows = slice(bl * 8, bl * 8 + 8)
            cols = slice(bl * 32, bl * 32 + 32)
            pen[:, rows, cols] = np.where(
                mc[16 * np.arange(NGl) + bl][:, None, :], 0.0, -30000.0)
        m = {
            "x": np.ascontiguousarray(h_neighbors[sl]),
            "hc": np.ascontiguousarray(h_center[sl]),
            "pen": pen,
            "w1s": w1s_, "nw1b": nw1b_, "wq": wq_, "wkt": wkt_,
            "wv": wv_, "wo": wo_, "s32": s32, "bvt": bvt, "bob": bob,
        }
        if not trivial:
            m["b1b"] = np.ascontiguousarray(
                np.broadcast_to(np.asarray(b1, f32), (128, H)))
            m["bqb"] = np.ascontiguousarray(
                np.broadcast_to(np.asarray(bq, f32), (128, H)))
            m["lgb"] = np.ascontiguousarray(
                np.broadcast_to(np.asarray(ln_g, f32), (128, H)))
            m["lbb"] = np.ascontiguousarray(
                np.broadcast_to(np.asarray(ln_b, f32), (128, H)))
        in_maps.append(m)
    return in_maps, trivial


_BUILD_CACHE = {}


def build_nc(B_local, NG_local, trivial_affine, gelu_fused=True,
             n_cores=N_CORES):
    """Build + compile the Bass module for a per-core batch of B_local."""
    key = (B_local, trivial_affine, gelu_fused, n_cores)
    if key in _BUILD_CACHE:
        return _BUILD_CACHE[key]
    nc = bacc.Bacc("TRN2", target_bir_lowering=False, debug=False,
                   num_devices=n_cores)

    def din(name, shape, dt):
        return nc.dram_tensor(name, list(shape), dt, kind="ExternalInput").ap()

    ins = {
        "x": din("x", (B_local, 32, 512), F32),
        "hc": din("hc", (B_local, 512), F32),
        "pen": din("pen", (NG_local, 128, 512), F32),
        "w1s": din("w1s", (4, 128, 512), BF16),
        "nw1b": din("nw1b", (4, 128, 512), BF16),
        "wq": din("wq", (4, 128, 512), BF16),
        "wkt": din("wkt", (4, 128, 512), BF16),
        "wv": din("wv", (4, 128, 512), BF16),
        "wo": din("wo", (4, 128, 512), BF16),
        "s32": din("s32", (32, 128, 128), BF16),
        "bvt": din("bvt", (4, 128, 1), F32),
        "bob": din("bob", (128, 512), F32),
    }
    if not trivial_affine:
        for nm in ("b1b", "bqb", "lgb", "lbb"):
            ins[nm] = din(nm, (128, 512), F32)
    outs = {"out": nc.dram_tensor("out", [B_local, 512], F32,
                                  kind="ExternalOutput").ap()}

    kfn = make_kernel(B_local, trivial_affine, gelu_fused)
    with tile.TileContext(nc) as tc:
        kfn(tc, outs, ins)
    nc.compile()
    _BUILD_CACHE[key] = nc
    return nc


def kernel(h_center, h_neighbors, W1, b1, ln_g, ln_b, Wq, bq, Wk, bk,
           Wv, bv, Wo, bo, neighbor_mask):
    B = h_neighbors.shape[0]
    Bl = B // N_CORES
    in_maps, trivial = prep_inputs(
        h_center, h_neighbors, W1, b1, ln_g, ln_b, Wq, bq, Wk, bk,
        Wv, bv, Wo, bo, neighbor_mask, n_cores=N_CORES)
    nc = build_nc(Bl, Bl // 16, trivial)
    res = run_bass_kernel_spmd(nc, in_maps, list(range(N_CORES)))
    return np.concatenate([r["out"] for r in res.results], axis=0).astype(
        np.float32)
